# revision 13
# baseline (speedup 1.0000x reference)
"""Trainium2 Bass kernel for nn_Conv2d_mvm (bit-streamed crossbar MVM conv).

Contract: kernel(**inputs) takes FULL unsharded inputs {x:[8,64,16,16] f32,
weight:[128,64,3,3] f32} and returns the FULL output [8,128,16,16] f32.

Sharding (8 cores): pixels P=2048 split 4 ways x crossbar-sign (pos/neg)
split 2 ways.  Core i: sign n=i//4, pixel quarter q=i%4 (512 pixels).
All cores run the identical SPMD program; sign is folded on the host.

v2 architecture (vs v1's two-matmul-pass + 2 quant ops per tile):
  * Columns are packed per (row-block r, weight-slice sl).  Weight slices
    that are all-zero for BOTH signs (top 3 slices for |w|<0.25 data) are
    skipped entirely -> 45 tiles instead of 72 per stream.
  * Pass-1 matmuls use fp8 DoubleRow perf mode: two 128-partition k-tiles
    (four 64-row crossbar row-blocks, block-diagonal) contract in a single
    instruction at 0.5 PE cycles per output row.
  * Because each psq tile's 128 partitions are exactly the 128 output
    channels of one (r, sl), the shift-add reduction matrix is a scaled
    identity -> no second matmul pass at all.  The slice weight 4^(7-sl)
    and stream weight +-2^s are folded into the quantization scale.
  * ADC emulation per (tile, stream) is exactly 2 vector ops:
      op1 (ACT/DVE/Pool): y = psq*(sgn*85/64*2^s*4w) + sgn*2^(s+23)*4w
          f32 RNE at the magic bias rounds R = round_half_even(col*255/192)
      op2 (DVE/Pool):     S_eng = (y - bias) + S_eng    [scalar_tensor_tensor]
          (y - bias) = sgn*2^s*4w*R exactly; S accumulates all (r,sl,s).
    Work is spread across the three vector engines by a static balance.
  * Device output = S_DVE + S_POOL; host applies (192/255)*2^-24 scaling,
    pos-neg, final fixed-point round/clip.
"""

import numpy as np
import ml_dtypes
from contextlib import ExitStack

# ---- problem constants (hardcoded; must match the reference) ----
B, C, H, W = 8, 64, 16, 16
O, KH, KW = 128, 3, 3
PAD = 1
OH = OW = 16
L = C * KH * KW            # 576
XBAR = 64
SLICE_NUM = 8              # 16-bit weights / 2-bit slices
STREAM_NUM = 16            # 16-bit inputs / 1-bit streams
NSTATES = 3
W_FRAC = 12
I_FRAC = 12
XR = 9                     # ceil(576/64) row blocks
P_TOTAL = B * OH * OW      # 2048
N_CORES = 8
P_CORE = P_TOTAL // 4      # 512 pixels per core (4-way pixel shard)

# row-block groups: (first rb, n_ktiles, partitions). Each DoubleRow matmul
# contracts 2 k-tiles; G0/G1 cover 4 row-blocks each, G2 covers rb 8 (+zeros).
GROUPS = [(0, 2, 128), (4, 2, 128), (8, 2, 64)]

# Per-(tile,stream) unit path split (tuned from trace).  Constraints found
# by probing walrus: every compute instruction fits ONE sync wait;
# scalar_tensor_tensor is DVE-only; Pool cannot touch PSUM.
#   P2: DVE magic-round -> DVE fused debias+accumulate into S_DVE
#   P3: DVE magic-round -> Pool debias op + Pool accumulate into S_POOL
#   P4: ACT magic-round -> Pool debias op + Pool accumulate into S_POOL
#   P1: ACT magic-round -> ACT debias->bf16 qp -> PE identity-matmul into acc
#   P5: DVE magic-round -> DVE debias->bf16 qp -> PE identity-matmul into acc
PATH_SPLIT = {"P2": 0, "P3": 0, "P4": 300, "P1": 100, "P5": 320}

_COMPILED = {}


# ------------------------- host-side preprocessing -------------------------

def _weight_cells(weight):
    """-> cells[2, 8, O, L] int8: per sign/slice crossbar cell values 0..3."""
    wf = weight.reshape(O, L)
    cells = np.zeros((2, SLICE_NUM, O, L), np.int8)
    for n, w_mag in enumerate((np.clip(wf, 0.0, None), np.abs(np.clip(wf, None, 0.0)))):
        w_int = np.clip(np.round(w_mag * 2.0**W_FRAC), 0, 2**16 - 1).astype(np.int64)
        for sl in range(SLICE_NUM):
            cells[n, sl] = ((w_int >> (2 * (SLICE_NUM - 1 - sl))) & NSTATES).astype(np.int8)
    return cells


def _nz_slices(cells):
    """Slices sl that are nonzero for either sign anywhere -> tile structure."""
    nz = [(cells[:, sl] != 0).any() for sl in range(SLICE_NUM)]
    return [sl for sl in range(SLICE_NUM) if nz[sl]]


def _prep_xb(cells, n, sls):
    """Per sign n: list per group of [P, T, 2, 128] fp8 stationaries.

    Tile t=(r_in_group, slc): lhsT[p, kt, m] = cells[n, sl, m, l] for
    l = 64*rb0 + 128*kt + p when that row lies in row-block r, else 0.
    """
    out = []
    for (rb0, nkt, part) in GROUPS:
        n_rb = min(4, XR - rb0) if part == 128 else 1
        T = n_rb * len(sls)
        xb = np.zeros((part, T, 2, 128), np.float32)
        t = 0
        for ri in range(n_rb):
            r = rb0 + ri
            kt, band = divmod(ri, 2) if part == 128 else (0, 0)
            for sl in sls:
                # rows of this rb live at partitions band*64..band*64+64, ktile kt
                cell = cells[n, sl][:, 64 * r:64 * r + 64]       # [O=128, 64]
                xb[64 * band:64 * band + 64, t, kt, :] = cell.T  # [64, 128]
                t += 1
        out.append(np.ascontiguousarray(xb.astype(ml_dtypes.float8_e4m3)))
    return out


def _prep_bits(x):
    """-> list per group of [P, 2, 16, 2048] fp8 bit-stream moving tiles."""
    xp = np.pad(x, ((0, 0), (0, 0), (PAD, PAD), (PAD, PAD)))
    patches = np.stack([xp[:, :, di:di + OH, dj:dj + OW]
                        for di in range(KH) for dj in range(KW)], axis=2)
    feat = patches.reshape(B, L, OH * OW).transpose(0, 2, 1).reshape(P_TOTAL, L)
    x_int = np.clip(np.round(feat * 2.0**I_FRAC), -2**15, 2**15 - 1).astype(np.int32)
    x_u = np.where(x_int < 0, x_int + 2**16, x_int).astype(np.uint16)
    shifts = np.arange(STREAM_NUM, dtype=np.int32)[None, None, :]
    bits = ((x_u[:, :, None] >> shifts) & 1).astype(np.float32)  # [P, L, 16]
    bits = bits.transpose(1, 2, 0)                               # [L, 16, P]
    out = []
    for (rb0, nkt, part) in GROUPS:
        bg = np.zeros((part, nkt, STREAM_NUM, P_TOTAL), np.float32)
        for kt in range(nkt):
            l0 = 64 * rb0 + part * kt
            if l0 >= L:
                continue
            rows = min(part, L - l0)
            bg[:rows, kt] = bits[l0:l0 + rows]
        out.append(np.ascontiguousarray(bg.astype(ml_dtypes.float8_e4m3)))
    return out


def _tile_list(sls):
    """[(g, t_in_group, sl)] enumeration matching _prep_xb layout."""
    tiles = []
    for g, (rb0, nkt, part) in enumerate(GROUPS):
        n_rb = min(4, XR - rb0) if part == 128 else 1
        t = 0
        for ri in range(n_rb):
            for sl in sls:
                tiles.append((g, t, sl))
                t += 1
    return tiles


def _engine_schedule(n_units):
    """Deterministic per-(tile,stream) path assignment matching PATH_SPLIT
    fractions, interleaved for pipeline overlap."""
    split = {k: v for k, v in PATH_SPLIT.items() if v > 0}
    total = sum(split.values())
    acc = {k: 0.0 for k in split}
    seq = []
    for i in range(n_units):
        for k in split:
            acc[k] += split[k] / total
        k = max(acc, key=lambda kk: acc[kk])
        acc[k] -= 1.0
        seq.append(k)
    return seq


# ------------------------------ bass program ------------------------------

def _build_nc(sls):
    import concourse.bass as bass
    import concourse.mybir as mybir
    import concourse.tile as tile

    f8 = mybir.dt.float8e4
    f32 = mybir.dt.float32

    tiles = _tile_list(sls)
    n_tiles = len(tiles)
    sched = _engine_schedule(n_tiles * STREAM_NUM)

    bf16 = mybir.dt.bfloat16
    nc = bass.Bass()
    bits_d, xb_d = [], []
    for g, (rb0, nkt, part) in enumerate(GROUPS):
        n_rb = min(4, XR - rb0) if part == 128 else 1
        T = n_rb * len(sls)
        bits_d.append(nc.dram_tensor(f"bits{g}", [part, nkt, STREAM_NUM, P_CORE],
                                     f8, kind="ExternalInput"))
        xb_d.append(nc.dram_tensor(f"xb{g}", [part, T, 2, 128], f8,
                                   kind="ExternalInput"))
    id_d = nc.dram_tensor("ident", [128, 128], bf16, kind="ExternalInput")
    out_d = nc.dram_tensor("acc_out", [128, P_CORE], f32, kind="ExternalOutput")

    e1_of = {"P2": "DVE", "P3": "DVE", "P4": "ACT", "P1": "ACT", "P5": "DVE"}
    pe_units = [i for i, p in enumerate(sched) if p in ("P1", "P5")]
    first_pe, last_pe = (pe_units[0], pe_units[-1]) if pe_units else (-1, -1)

    with ExitStack() as ctx:
        tc = ctx.enter_context(tile.TileContext(nc))
        singles = ctx.enter_context(tc.tile_pool(name="singles", bufs=1))
        psq_pools = {
            "ACT": ctx.enter_context(tc.tile_pool(name="psqA", bufs=3, space="PSUM")),
            "DVE": ctx.enter_context(tc.tile_pool(name="psqV", bufs=3, space="PSUM")),
        }
        acc_pool = ctx.enter_context(tc.tile_pool(name="accp", bufs=1, space="PSUM"))
        y_pools = {p: ctx.enter_context(tc.tile_pool(name=f"y{p}", bufs=2))
                   for p in PATH_SPLIT if PATH_SPLIT[p] > 0}
        qp_pools = {p: ctx.enter_context(tc.tile_pool(name=f"qp{p}", bufs=3))
                    for p in ("P1", "P5") if PATH_SPLIT[p] > 0}
        y2_pool = ctx.enter_context(tc.tile_pool(name="y2p", bufs=2))

        xb_sb, bits_sb = [], []
        for g in range(len(GROUPS)):
            xb = singles.tile(list(xb_d[g].shape), f8, name=f"xbs{g}")
            nc.default_dma_engine.dma_start(out=xb[:], in_=xb_d[g][:, :, :, :])
            bsb = singles.tile(list(bits_d[g].shape), f8, name=f"bsb{g}")
            nc.default_dma_engine.dma_start(out=bsb[:], in_=bits_d[g][:, :, :, :])
            xb_sb.append(xb)
            bits_sb.append(bsb)
        id_sb = singles.tile([128, 128], bf16, name="idsb")
        nc.default_dma_engine.dma_start(out=id_sb[:], in_=id_d[:, :])

        s_dve = singles.tile([128, P_CORE], f32, name="sdve")
        s_pool = singles.tile([128, P_CORE], f32, name="spool")
        nc.vector.memset(s_dve[:, :], 0.0)
        nc.gpsimd.memset(s_pool[:, :], 0.0)
        acc = acc_pool.tile([128, P_CORE], f32, name="accps")

        unit = 0
        for (g, t, sl) in tiles:
            for s in range(STREAM_NUM):
                path = sched[unit]
                e1 = e1_of[path]
                sgn = -1.0 if s == STREAM_NUM - 1 else 1.0
                w4 = 2.0 ** (2 * (SLICE_NUM - 1 - sl))
                scale = float(np.float32(sgn * (85.0 / 64.0) * 2.0**s * w4))
                bias = float(np.float32(sgn * 2.0**(s + 23) * w4))

                psq = psq_pools[e1].tile([128, P_CORE], f32, tag=f"psq{e1}",
                                         name=f"psq{e1}")
                nc.tensor.matmul(psq[:, :], xb_sb[g][:, t, :, :],
                                 bits_sb[g][:, :, s, :], start=True, stop=True,
                                 perf_mode=mybir.MatmulPerfMode.DoubleRow)

                y = y_pools[path].tile([128, P_CORE], f32, tag=f"y{path}",
                                       name=f"y{path}")
                if e1 == "ACT":
                    nc.scalar.activation(y[:, :], psq[:, :],
                                         mybir.ActivationFunctionType.Copy,
                                         bias=bias, scale=scale)
                else:
                    nc.vector.tensor_scalar(y[:, :], psq[:, :], scale, bias,
                                            mybir.AluOpType.mult,
                                            mybir.AluOpType.add)
                if path == "P2":
                    nc.vector.scalar_tensor_tensor(
                        s_dve[:, :], y[:, :], -bias, s_dve[:, :],
                        mybir.AluOpType.add, mybir.AluOpType.add)
                elif path in ("P3", "P4"):
                    y2 = y2_pool.tile([128, P_CORE], f32, tag="y2", name="y2")
                    nc.gpsimd.tensor_scalar(y2[:, :], y[:, :], -bias, None,
                                            mybir.AluOpType.add)
                    nc.gpsimd.tensor_tensor(s_pool[:, :], y2[:, :],
                                            s_pool[:, :], mybir.AluOpType.add)
                else:  # P1 / P5: debias -> bf16 qp -> PE identity accumulate
                    qp = qp_pools[path].tile([128, P_CORE], bf16,
                                             tag=f"qp{path}", name=f"qp{path}")
                    if e1 == "ACT":
                        nc.scalar.activation(qp[:, :], y[:, :],
                                             mybir.ActivationFunctionType.Copy,
                                             bias=-bias, scale=1.0)
                    else:
                        nc.vector.tensor_scalar(qp[:, :], y[:, :], -bias, None,
                                                mybir.AluOpType.add)
                    nc.tensor.matmul(acc[:, :], id_sb[:, :], qp[:, :],
                                     start=(unit == first_pe),
                                     stop=(unit == last_pe))
                unit += 1

        out_sb = singles.tile([128, P_CORE], f32, name="outsb")
        nc.vector.tensor_tensor(out_sb[:, :], s_dve[:, :], s_pool[:, :],
                                mybir.AluOpType.add)
        if pe_units:
            acc_sb = singles.tile([128, P_CORE], f32, name="accsb")
            nc.vector.tensor_copy(acc_sb[:, :], acc[:, :])
            nc.vector.tensor_tensor(out_sb[:, :], out_sb[:, :], acc_sb[:, :],
                                    mybir.AluOpType.add)
        nc.default_dma_engine.dma_start(out=out_d[:, :], in_=out_sb[:, :])

    _strip_own_engine_waits(nc, mybir)
    _move_surplus_waits_to_pe(nc, mybir)
    return nc


def _move_surplus_waits_to_pe(nc, mybir):
    """The ACT/DVE quantize op of each unit waits on {PE: psq ready} and
    {consumer engine: y buffer free}.  The Activation (and possibly TS) ISA
    sync structs hold only one wait, so move every non-PE wait onto the
    Ldweights (or Matmult) of the same unit: those run strictly earlier on
    the in-order PE queue than the quantize op (which RAW-waits its matmul),
    so the guarantee still holds when the quantize op issues."""
    for f in nc.m.functions:
        for b in f.blocks:
            insts = b.instructions
            # index: psq memref -> Ldweights of the producing matmul
            producer = {}
            for j, pj in enumerate(insts):
                if (type(pj).__name__ == "InstMatmult" and pj.outs
                        and str(pj.outs[0].memref).startswith("psq")
                        and j > 0
                        and type(insts[j - 1]).__name__ == "InstLdweights"):
                    producer[str(pj.outs[0].memref)] = insts[j - 1]

            def add_waits(target, waits):
                tsi = getattr(target, "sync_info", None)
                twaits = list(tsi.on_wait or []) if tsi else []
                tupd = list(tsi.on_update or []) if tsi else []
                target.sync_info = mybir.SyncInfo(
                    on_wait=twaits + waits, on_update=tupd)

            # quantize ops: keep only the PE wait, surplus -> unit's Ldweights
            for inst in insts:
                tn = type(inst).__name__
                if tn not in ("InstActivation", "InstTensorScalarPtr"):
                    continue
                si = getattr(inst, "sync_info", None)
                if si is None or not si.on_wait or len(si.on_wait) <= 1:
                    continue
                try:
                    src = str(inst.ins[0].memref)
                except (AttributeError, IndexError):
                    continue
                if not src.startswith("psq"):
                    continue
                keep = [w for w in si.on_wait if str(w.ant_name).startswith("PE")]
                move = [w for w in si.on_wait if not str(w.ant_name).startswith("PE")]
                target = producer.get(src)
                if not move or target is None:
                    continue
                add_waits(target, move)
                inst.sync_info = mybir.SyncInfo(
                    on_wait=keep, on_update=list(si.on_update or []))
            # smooth the PE queue to <=1 wait per instruction: spill surplus
            # waits to the nearest EARLIER PE instruction with a free slot
            # (stalls strictly earlier on the in-order queue -> always safe
            # here: the spilled sems' producers are many units older).
            pe_idx = [j for j, pj in enumerate(insts)
                      if type(pj).__name__ in ("InstMatmult", "InstLdweights")]
            for k, j in enumerate(pe_idx):
                pj = insts[j]
                si = getattr(pj, "sync_info", None)
                if si is None or not si.on_wait or len(si.on_wait) <= 1:
                    continue
                keep, surplus = list(si.on_wait[:1]), list(si.on_wait[1:])
                for kk in range(k - 1, max(-1, k - 8), -1):
                    tgt = insts[pe_idx[kk]]
                    tsi = getattr(tgt, "sync_info", None)
                    if tsi is None or not tsi.on_wait:
                        add_waits(tgt, [surplus.pop()])
                        if not surplus:
                            break
                keep += surplus  # anything unplaced stays (compile will flag)
                pj.sync_info = mybir.SyncInfo(
                    on_wait=keep, on_update=list(si.on_update or []))


def _strip_own_engine_waits(nc, mybir):
    """Drop redundant same-engine semaphore waits (compute engines execute
    their queue serially, so ordering vs. their own past instructions is
    implicit).  Walrus's per-instruction sync structs have very few wait
    slots and reject Tile's extra own-engine waits."""
    eng_prefix = {
        "EngineType.PE": "PE",
        "EngineType.Activation": "Activation",
        "EngineType.DVE": "DVE",
        "EngineType.Pool": "Pool",
    }
    # The tail drain waits on every engine + every DMA queue, exceeding the
    # CTRL struct's wait slots.  All but the final DVE->DRAM chain are implied
    # transitively, so keep only the output DMA queue's semaphore.
    last_dma_sems = set()
    for f in nc.m.functions:
        for b in f.blocks:
            for inst in b.instructions:
                if type(inst).__name__ == "InstDMACopy" and inst.sync_info:
                    last_dma_sems = {str(w.ant_name)
                                     for w in (inst.sync_info.on_update or [])}
    for f in nc.m.functions:
        for b in f.blocks:
            for inst in b.instructions:
                si = getattr(inst, "sync_info", None)
                if (type(inst).__name__ == "InstDrain" and si and si.on_wait
                        and len(si.on_wait) > 2):
                    kept = [w for w in si.on_wait
                            if str(w.ant_name) in last_dma_sems]
                    inst.sync_info = mybir.SyncInfo(
                        on_wait=kept, on_update=list(si.on_update or []))
    for f in nc.m.functions:
        for b in f.blocks:
            for inst in b.instructions:
                si = getattr(inst, "sync_info", None)
                if si is None or not si.on_wait:
                    continue
                pfx = eng_prefix.get(str(getattr(inst, "engine", None)))
                if pfx is None:
                    continue
                kept = [w for w in si.on_wait
                        if not str(w.ant_name).startswith(pfx + "_")]
                if len(kept) != len(si.on_wait):
                    inst.sync_info = mybir.SyncInfo(
                        on_wait=kept, on_update=list(si.on_update or []))


# ------------------------------- entry point -------------------------------

def _make_in_maps(x, weight, sls, cells):
    bits = _prep_bits(x)                       # per group [P, 2, 16, 2048]
    xb = [_prep_xb(cells, n, sls) for n in (0, 1)]
    ident = np.ascontiguousarray(np.eye(128, dtype=np.float32)
                                 .astype(ml_dtypes.bfloat16))
    in_maps = []
    for core in range(N_CORES):
        n, q = core // 4, core % 4
        im = {"ident": ident}
        for g in range(len(GROUPS)):
            im[f"bits{g}"] = np.ascontiguousarray(
                bits[g][:, :, :, q * P_CORE:(q + 1) * P_CORE])
            im[f"xb{g}"] = xb[n][g]
        in_maps.append(im)
    return in_maps


def _postprocess(accs):
    """accs: list of 8 [128, 512] f32 arrays (core order) -> [8,128,16,16]."""
    acc_pos = np.concatenate([accs[q] for q in range(4)], axis=1)       # [128,2048]
    acc_neg = np.concatenate([accs[4 + q] for q in range(4)], axis=1)
    d32 = np.float32(192.0 / 255.0)
    out = ((acc_pos - acc_neg).astype(np.float32) * d32).astype(np.float32)
    out = out * np.float32(2.0**-24)
    amax = np.float32((2**15 - 1) / 2.0**12)
    out = np.clip(np.round(out * np.float32(4096.0)) / np.float32(4096.0),
                  -amax, amax).astype(np.float32)
    return np.ascontiguousarray(
        out.reshape(O, B, OH, OW).transpose(1, 0, 2, 3))


def run_on_hw(x, weight, trace=False):
    from concourse.bass_utils import run_bass_kernel_spmd
    x = np.asarray(x, np.float32)
    weight = np.asarray(weight, np.float32)
    cells = _weight_cells(weight)
    sls = _nz_slices(cells)
    key = tuple(sls)
    if key not in _COMPILED:
        _COMPILED[key] = _build_nc(sls)
    nc = _COMPILED[key]
    in_maps = _make_in_maps(x, weight, sls, cells)
    res = run_bass_kernel_spmd(nc, in_maps, list(range(N_CORES)), trace=trace)
    accs = [np.asarray(res.results[i]["acc_out"], np.float32)
            for i in range(N_CORES)]
    return _postprocess(accs), res


def kernel(x, weight):
    out, _ = run_on_hw(x, weight, trace=False)
    return out


# revision 18
# speedup vs baseline: 4.0434x; 4.0434x over previous
"""Trainium2 Bass kernel for nn_Conv2d_mvm (bit-streamed crossbar MVM conv).

Contract: kernel(**inputs) takes FULL unsharded inputs {x:[8,64,16,16] f32,
weight:[128,64,3,3] f32} and returns the FULL output [8,128,16,16] f32.

Sharding (8 cores): pixels P=2048 split 4 ways x crossbar-sign (pos/neg)
split 2 ways.  Core i: sign n=i//4, pixel quarter q=i%4 (512 pixels).
All cores run the identical SPMD program; sign is folded on the host.

v2 architecture (vs v1's two-matmul-pass + 2 quant ops per tile):
  * Columns are packed per (row-block r, weight-slice sl).  Weight slices
    that are all-zero for BOTH signs (top 3 slices for |w|<0.25 data) are
    skipped entirely -> 45 tiles instead of 72 per stream.
  * Pass-1 matmuls use fp8 DoubleRow perf mode: two 128-partition k-tiles
    (four 64-row crossbar row-blocks, block-diagonal) contract in a single
    instruction at 0.5 PE cycles per output row.
  * Because each psq tile's 128 partitions are exactly the 128 output
    channels of one (r, sl), the shift-add reduction matrix is a scaled
    identity -> no second matmul pass at all.  The slice weight 4^(7-sl)
    and stream weight +-2^s are folded into the quantization scale.
  * ADC emulation per (tile, stream) is exactly 2 vector ops:
      op1 (ACT/DVE/Pool): y = psq*(sgn*85/64*2^s*4w) + sgn*2^(s+23)*4w
          f32 RNE at the magic bias rounds R = round_half_even(col*255/192)
      op2 (DVE/Pool):     S_eng = (y - bias) + S_eng    [scalar_tensor_tensor]
          (y - bias) = sgn*2^s*4w*R exactly; S accumulates all (r,sl,s).
    Work is spread across the three vector engines by a static balance.
  * Device output = S_DVE + S_POOL; host applies (192/255)*2^-24 scaling,
    pos-neg, final fixed-point round/clip.
"""

import numpy as np
import ml_dtypes
from contextlib import ExitStack

# ---- problem constants (hardcoded; must match the reference) ----
B, C, H, W = 8, 64, 16, 16
O, KH, KW = 128, 3, 3
PAD = 1
OH = OW = 16
L = C * KH * KW            # 576
XBAR = 64
SLICE_NUM = 8              # 16-bit weights / 2-bit slices
STREAM_NUM = 16            # 16-bit inputs / 1-bit streams
NSTATES = 3
W_FRAC = 12
I_FRAC = 12
XR = 9                     # ceil(576/64) row blocks
P_TOTAL = B * OH * OW      # 2048
N_CORES = 8
P_CORE = P_TOTAL // 4      # 512 pixels per core (4-way pixel shard)

# row-block groups: (first rb, n_ktiles, partitions). Each DoubleRow matmul
# contracts 2 k-tiles; G0/G1 cover 4 row-blocks each, G2 covers rb 8 (+zeros).
GROUPS = [(0, 2, 128), (4, 2, 128), (8, 2, 64)]

# Per-(tile,stream) unit path split (tuned from trace).  Constraints found
# by probing walrus + hw: every compute instruction fits ONE sync wait;
# scalar_tensor_tensor is DVE-only; Pool cannot touch PSUM; Pool
# tensor_scalar is ~7us (useless) but tensor_tensor is ~1.3us; measured
# per-[128,512]-op: ACT ~0.66us, DVE f32 ~0.63us, DVE ->16bit ~0.34us,
# PE matmul ~0.49us (DoubleRow gives no real-hw gain).
#   P2:  DVE magic-round -> DVE fused debias+accumulate into S_DVE
#   P6:  ACT magic-round -> DVE fused debias+accumulate into S_DVE
#   P7:  ACT magic-round -> DVE debias->bf16 qp -> Pool TT add into S_POOL
#   P8a: ACT magic-round -> ACT debias->bf16 qp -> PE identity-matmul acc
#   P8b: DVE magic-round -> DVE debias->bf16 qp -> PE identity-matmul acc
PATH_SPLIT = {"P2": 0, "P6": 269, "P7": 315, "P8a": 0, "P8b": 136}

_COMPILED = {}


# ------------------------- host-side preprocessing -------------------------

def _weight_cells(weight):
    """-> cells[2, 8, O, L] int8: per sign/slice crossbar cell values 0..3."""
    wf = weight.reshape(O, L)
    cells = np.zeros((2, SLICE_NUM, O, L), np.int8)
    for n, w_mag in enumerate((np.clip(wf, 0.0, None), np.abs(np.clip(wf, None, 0.0)))):
        w_int = np.clip(np.round(w_mag * 2.0**W_FRAC), 0, 2**16 - 1).astype(np.int64)
        for sl in range(SLICE_NUM):
            cells[n, sl] = ((w_int >> (2 * (SLICE_NUM - 1 - sl))) & NSTATES).astype(np.int8)
    return cells


def _nz_slices(cells):
    """Slices sl that are nonzero for either sign anywhere -> tile structure."""
    nz = [(cells[:, sl] != 0).any() for sl in range(SLICE_NUM)]
    return [sl for sl in range(SLICE_NUM) if nz[sl]]


def _prep_xb(cells, n, sls):
    """Per sign n: list per group of [P, T, 2, 128] fp8 stationaries.

    Tile t=(r_in_group, slc): lhsT[p, kt, m] = cells[n, sl, m, l] for
    l = 64*rb0 + 128*kt + p when that row lies in row-block r, else 0.
    """
    out = []
    for (rb0, nkt, part) in GROUPS:
        n_rb = min(4, XR - rb0) if part == 128 else 1
        T = n_rb * len(sls)
        xb = np.zeros((part, T, 2, 128), np.float32)
        t = 0
        for ri in range(n_rb):
            r = rb0 + ri
            kt, band = divmod(ri, 2) if part == 128 else (0, 0)
            for sl in sls:
                # rows of this rb live at partitions band*64..band*64+64, ktile kt
                cell = cells[n, sl][:, 64 * r:64 * r + 64]       # [O=128, 64]
                xb[64 * band:64 * band + 64, t, kt, :] = cell.T  # [64, 128]
                t += 1
        out.append(np.ascontiguousarray(xb.astype(ml_dtypes.float8_e4m3)))
    return out


def _prep_bits(x):
    """-> list per group of [P, 2, 16, 2048] fp8 bit-stream moving tiles."""
    xp = np.pad(x, ((0, 0), (0, 0), (PAD, PAD), (PAD, PAD)))
    patches = np.stack([xp[:, :, di:di + OH, dj:dj + OW]
                        for di in range(KH) for dj in range(KW)], axis=2)
    feat = patches.reshape(B, L, OH * OW).transpose(0, 2, 1).reshape(P_TOTAL, L)
    x_int = np.clip(np.round(feat * 2.0**I_FRAC), -2**15, 2**15 - 1).astype(np.int32)
    x_u = np.where(x_int < 0, x_int + 2**16, x_int).astype(np.uint16)
    shifts = np.arange(STREAM_NUM, dtype=np.int32)[None, None, :]
    bits = ((x_u[:, :, None] >> shifts) & 1).astype(np.float32)  # [P, L, 16]
    bits = bits.transpose(1, 2, 0)                               # [L, 16, P]
    out = []
    for (rb0, nkt, part) in GROUPS:
        bg = np.zeros((part, nkt, STREAM_NUM, P_TOTAL), np.float32)
        for kt in range(nkt):
            l0 = 64 * rb0 + part * kt
            if l0 >= L:
                continue
            rows = min(part, L - l0)
            bg[:rows, kt] = bits[l0:l0 + rows]
        out.append(np.ascontiguousarray(bg.astype(ml_dtypes.float8_e4m3)))
    return out


def _tile_list(sls):
    """[(g, t_in_group, sl)] enumeration matching _prep_xb layout."""
    tiles = []
    for g, (rb0, nkt, part) in enumerate(GROUPS):
        n_rb = min(4, XR - rb0) if part == 128 else 1
        t = 0
        for ri in range(n_rb):
            for sl in sls:
                tiles.append((g, t, sl))
                t += 1
    return tiles


def _engine_schedule(n_units):
    """Deterministic per-(tile,stream) path assignment matching PATH_SPLIT
    fractions, interleaved for pipeline overlap."""
    split = {k: v for k, v in PATH_SPLIT.items() if v > 0}
    total = sum(split.values())
    acc = {k: 0.0 for k in split}
    seq = []
    for i in range(n_units):
        for k in split:
            acc[k] += split[k] / total
        k = max(acc, key=lambda kk: acc[kk])
        acc[k] -= 1.0
        seq.append(k)
    return seq


# ------------------------------ bass program ------------------------------

def _build_nc(sls):
    import concourse.bass as bass
    import concourse.mybir as mybir
    import concourse.tile as tile

    f8 = mybir.dt.float8e4
    f32 = mybir.dt.float32

    tiles = _tile_list(sls)
    n_tiles = len(tiles)
    sched = _engine_schedule(n_tiles * STREAM_NUM)

    bf16 = mybir.dt.bfloat16
    nc = bass.Bass()
    bits_d, xb_d = [], []
    for g, (rb0, nkt, part) in enumerate(GROUPS):
        n_rb = min(4, XR - rb0) if part == 128 else 1
        T = n_rb * len(sls)
        bits_d.append(nc.dram_tensor(f"bits{g}", [part, nkt, STREAM_NUM, P_CORE],
                                     f8, kind="ExternalInput"))
        xb_d.append(nc.dram_tensor(f"xb{g}", [part, T, 2, 128], f8,
                                   kind="ExternalInput"))
    id_d = nc.dram_tensor("ident", [128, 128], bf16, kind="ExternalInput")
    out_d = nc.dram_tensor("acc_out", [128, P_CORE], f32, kind="ExternalOutput")

    e1_of = {"P2": "DVE", "P6": "ACT", "P7": "ACT", "P8a": "ACT", "P8b": "DVE"}
    pe_units = [i for i, p in enumerate(sched) if p in ("P8a", "P8b")]
    first_pe, last_pe = (pe_units[0], pe_units[-1]) if pe_units else (-1, -1)

    with ExitStack() as ctx:
        tc = ctx.enter_context(tile.TileContext(nc))
        singles = ctx.enter_context(tc.tile_pool(name="singles", bufs=1))
        psq_pools = {
            "ACT": ctx.enter_context(tc.tile_pool(name="psqA", bufs=3, space="PSUM")),
            "DVE": ctx.enter_context(tc.tile_pool(name="psqV", bufs=3, space="PSUM")),
        }
        acc_pool = ctx.enter_context(tc.tile_pool(name="accp", bufs=1, space="PSUM"))
        y_pools = {p: ctx.enter_context(tc.tile_pool(name=f"y{p}", bufs=4))
                   for p in PATH_SPLIT if PATH_SPLIT[p] > 0}
        qp_pools = {p: ctx.enter_context(
                        tc.tile_pool(name=f"qp{p}", bufs=6 if p == "P7" else 3))
                    for p in ("P7", "P8a", "P8b") if PATH_SPLIT[p] > 0}

        xb_sb, bits_sb = [], []
        for g in range(len(GROUPS)):
            xb = singles.tile(list(xb_d[g].shape), f8, name=f"xbs{g}")
            nc.default_dma_engine.dma_start(out=xb[:], in_=xb_d[g][:, :, :, :])
            bsb = singles.tile(list(bits_d[g].shape), f8, name=f"bsb{g}")
            nc.default_dma_engine.dma_start(out=bsb[:], in_=bits_d[g][:, :, :, :])
            xb_sb.append(xb)
            bits_sb.append(bsb)
        id_sb = singles.tile([128, 128], bf16, name="idsb")
        nc.default_dma_engine.dma_start(out=id_sb[:], in_=id_d[:, :])

        s_dve = singles.tile([128, P_CORE], f32, name="sdve")
        s_pool = singles.tile([128, P_CORE], f32, name="spool")
        nc.vector.memset(s_dve[:, :], 0.0)
        nc.gpsimd.memset(s_pool[:, :], 0.0)
        acc = acc_pool.tile([128, P_CORE], f32, name="accps")

        unit = 0
        for (g, t, sl) in tiles:
            for s in range(STREAM_NUM):
                path = sched[unit]
                e1 = e1_of[path]
                sgn = -1.0 if s == STREAM_NUM - 1 else 1.0
                w4 = 2.0 ** (2 * (SLICE_NUM - 1 - sl))
                scale = float(np.float32(sgn * (85.0 / 64.0) * 2.0**s * w4))
                bias = float(np.float32(sgn * 2.0**(s + 23) * w4))

                psq = psq_pools[e1].tile([128, P_CORE], f32, tag=f"psq{e1}",
                                         name=f"psq{e1}")
                nc.tensor.matmul(psq[:, :], xb_sb[g][:, t, :, :],
                                 bits_sb[g][:, :, s, :], start=True, stop=True,
                                 perf_mode=mybir.MatmulPerfMode.DoubleRow)

                y = y_pools[path].tile([128, P_CORE], f32, tag=f"y{path}",
                                       name=f"y{path}")
                if e1 == "ACT":
                    nc.scalar.activation(y[:, :], psq[:, :],
                                         mybir.ActivationFunctionType.Copy,
                                         bias=bias, scale=scale)
                else:
                    nc.vector.tensor_scalar(y[:, :], psq[:, :], scale, bias,
                                            mybir.AluOpType.mult,
                                            mybir.AluOpType.add)
                if path in ("P2", "P6"):
                    nc.vector.scalar_tensor_tensor(
                        s_dve[:, :], y[:, :], -bias, s_dve[:, :],
                        mybir.AluOpType.add, mybir.AluOpType.add)
                elif path == "P7":
                    qp = qp_pools[path].tile([128, P_CORE], bf16,
                                             tag="qpP7", name="qpP7")
                    nc.vector.tensor_scalar(qp[:, :], y[:, :], -bias, None,
                                            mybir.AluOpType.add)
                    nc.gpsimd.tensor_tensor(s_pool[:, :], qp[:, :],
                                            s_pool[:, :], mybir.AluOpType.add)
                else:  # P8a / P8b: debias -> bf16 qp -> PE identity accumulate
                    qp = qp_pools[path].tile([128, P_CORE], bf16,
                                             tag=f"qp{path}", name=f"qp{path}")
                    if e1 == "ACT":
                        nc.scalar.activation(qp[:, :], y[:, :],
                                             mybir.ActivationFunctionType.Copy,
                                             bias=-bias, scale=1.0)
                    else:
                        nc.vector.tensor_scalar(qp[:, :], y[:, :], -bias, None,
                                                mybir.AluOpType.add)
                    nc.tensor.matmul(acc[:, :], id_sb[:, :], qp[:, :],
                                     start=(unit == first_pe),
                                     stop=(unit == last_pe))
                unit += 1

        out_sb = singles.tile([128, P_CORE], f32, name="outsb")
        nc.vector.tensor_tensor(out_sb[:, :], s_dve[:, :], s_pool[:, :],
                                mybir.AluOpType.add)
        if pe_units:
            acc_sb = singles.tile([128, P_CORE], f32, name="accsb")
            nc.vector.tensor_copy(acc_sb[:, :], acc[:, :])
            nc.vector.tensor_tensor(out_sb[:, :], out_sb[:, :], acc_sb[:, :],
                                    mybir.AluOpType.add)
        nc.default_dma_engine.dma_start(out=out_d[:, :], in_=out_sb[:, :])

    _strip_own_engine_waits(nc, mybir)
    _move_surplus_waits_to_pe(nc, mybir)
    return nc


def _move_surplus_waits_to_pe(nc, mybir):
    """The ACT/DVE quantize op of each unit waits on {PE: psq ready} and
    {consumer engine: y buffer free}.  The Activation (and possibly TS) ISA
    sync structs hold only one wait, so move every non-PE wait onto the
    Ldweights (or Matmult) of the same unit: those run strictly earlier on
    the in-order PE queue than the quantize op (which RAW-waits its matmul),
    so the guarantee still holds when the quantize op issues."""
    for f in nc.m.functions:
        for b in f.blocks:
            insts = b.instructions
            # index: psq memref -> Ldweights of the producing matmul
            producer = {}
            for j, pj in enumerate(insts):
                if (type(pj).__name__ == "InstMatmult" and pj.outs
                        and str(pj.outs[0].memref).startswith("psq")
                        and j > 0
                        and type(insts[j - 1]).__name__ == "InstLdweights"):
                    producer[str(pj.outs[0].memref)] = insts[j - 1]

            def add_waits(target, waits):
                tsi = getattr(target, "sync_info", None)
                twaits = list(tsi.on_wait or []) if tsi else []
                tupd = list(tsi.on_update or []) if tsi else []
                target.sync_info = mybir.SyncInfo(
                    on_wait=twaits + waits, on_update=tupd)

            # map y tiles back to the psq (hence Ldweights) of their unit
            psq_of_y = {}
            for inst in insts:
                if type(inst).__name__ in ("InstActivation", "InstTensorScalarPtr"):
                    try:
                        src = str(inst.ins[0].memref)
                        dst = str(inst.outs[0].memref)
                    except (AttributeError, IndexError):
                        continue
                    if src.startswith("psq") and dst.startswith("y"):
                        psq_of_y[dst] = src

            # quantize/debias ops: keep the RAW-producer wait, move buffer
            # WAR waits (Pool/PE consumers / cross-engine) to the Ldweights
            for inst in insts:
                tn = type(inst).__name__
                if tn not in ("InstActivation", "InstTensorScalarPtr"):
                    continue
                si = getattr(inst, "sync_info", None)
                if si is None or not si.on_wait or len(si.on_wait) <= 1:
                    continue
                try:
                    src = str(inst.ins[0].memref)
                except (AttributeError, IndexError):
                    continue
                if src.startswith("psq"):
                    keep = [w for w in si.on_wait
                            if str(w.ant_name).startswith("PE")]
                    move = [w for w in si.on_wait
                            if not str(w.ant_name).startswith("PE")]
                    target = producer.get(src)
                elif src.startswith("y") and src in psq_of_y:
                    keep = [w for w in si.on_wait
                            if not str(w.ant_name).startswith(("PE", "Pool"))]
                    move = [w for w in si.on_wait
                            if str(w.ant_name).startswith(("PE", "Pool"))]
                    target = producer.get(psq_of_y[src])
                else:
                    continue
                if not move or target is None:
                    continue
                add_waits(target, move)
                inst.sync_info = mybir.SyncInfo(
                    on_wait=keep, on_update=list(si.on_update or []))
            # smooth the PE queue to <=1 wait per instruction: spill surplus
            # waits to the nearest EARLIER PE instruction with a free slot
            # (stalls strictly earlier on the in-order queue -> always safe
            # here: the spilled sems' producers are many units older).
            pe_idx = [j for j, pj in enumerate(insts)
                      if type(pj).__name__ in ("InstMatmult", "InstLdweights")]
            for k, j in enumerate(pe_idx):
                pj = insts[j]
                si = getattr(pj, "sync_info", None)
                if si is None or not si.on_wait or len(si.on_wait) <= 1:
                    continue
                keep, surplus = list(si.on_wait[:1]), list(si.on_wait[1:])
                unplaced = []
                for w in surplus:
                    # spill distance bounded by how long ago the wait's
                    # producer ran (deeper pools -> farther back is safe)
                    win = 26 if str(w.ant_name).startswith("Pool") else 14
                    placed = False
                    for kk in range(k - 1, max(-1, k - win), -1):
                        tgt = insts[pe_idx[kk]]
                        tsi = getattr(tgt, "sync_info", None)
                        if tsi is None or not tsi.on_wait:
                            add_waits(tgt, [w])
                            placed = True
                            break
                    if not placed:
                        unplaced.append(w)
                keep += unplaced  # anything unplaced stays (compile will flag)
                pj.sync_info = mybir.SyncInfo(
                    on_wait=keep, on_update=list(si.on_update or []))


def _strip_own_engine_waits(nc, mybir):
    """Drop redundant same-engine semaphore waits (compute engines execute
    their queue serially, so ordering vs. their own past instructions is
    implicit).  Walrus's per-instruction sync structs have very few wait
    slots and reject Tile's extra own-engine waits."""
    eng_prefix = {
        "EngineType.PE": "PE",
        "EngineType.Activation": "Activation",
        "EngineType.DVE": "DVE",
        "EngineType.Pool": "Pool",
    }
    # The tail drain waits on every engine + every DMA queue, exceeding the
    # CTRL struct's wait slots.  All but the final DVE->DRAM chain are implied
    # transitively, so keep only the output DMA queue's semaphore.
    last_dma_sems = set()
    for f in nc.m.functions:
        for b in f.blocks:
            for inst in b.instructions:
                if type(inst).__name__ == "InstDMACopy" and inst.sync_info:
                    last_dma_sems = {str(w.ant_name)
                                     for w in (inst.sync_info.on_update or [])}
    for f in nc.m.functions:
        for b in f.blocks:
            for inst in b.instructions:
                si = getattr(inst, "sync_info", None)
                if (type(inst).__name__ == "InstDrain" and si and si.on_wait
                        and len(si.on_wait) > 2):
                    kept = [w for w in si.on_wait
                            if str(w.ant_name) in last_dma_sems]
                    inst.sync_info = mybir.SyncInfo(
                        on_wait=kept, on_update=list(si.on_update or []))
    for f in nc.m.functions:
        for b in f.blocks:
            for inst in b.instructions:
                si = getattr(inst, "sync_info", None)
                if si is None or not si.on_wait:
                    continue
                pfx = eng_prefix.get(str(getattr(inst, "engine", None)))
                if pfx is None:
                    continue
                kept = [w for w in si.on_wait
                        if not str(w.ant_name).startswith(pfx + "_")]
                if len(kept) != len(si.on_wait):
                    inst.sync_info = mybir.SyncInfo(
                        on_wait=kept, on_update=list(si.on_update or []))


# ------------------------------- entry point -------------------------------

def _make_in_maps(x, weight, sls, cells):
    bits = _prep_bits(x)                       # per group [P, 2, 16, 2048]
    xb = [_prep_xb(cells, n, sls) for n in (0, 1)]
    ident = np.ascontiguousarray(np.eye(128, dtype=np.float32)
                                 .astype(ml_dtypes.bfloat16))
    in_maps = []
    for core in range(N_CORES):
        n, q = core // 4, core % 4
        im = {"ident": ident}
        for g in range(len(GROUPS)):
            im[f"bits{g}"] = np.ascontiguousarray(
                bits[g][:, :, :, q * P_CORE:(q + 1) * P_CORE])
            im[f"xb{g}"] = xb[n][g]
        in_maps.append(im)
    return in_maps


def _postprocess(accs):
    """accs: list of 8 [128, 512] f32 arrays (core order) -> [8,128,16,16]."""
    acc_pos = np.concatenate([accs[q] for q in range(4)], axis=1)       # [128,2048]
    acc_neg = np.concatenate([accs[4 + q] for q in range(4)], axis=1)
    d32 = np.float32(192.0 / 255.0)
    out = ((acc_pos - acc_neg).astype(np.float32) * d32).astype(np.float32)
    out = out * np.float32(2.0**-24)
    amax = np.float32((2**15 - 1) / 2.0**12)
    out = np.clip(np.round(out * np.float32(4096.0)) / np.float32(4096.0),
                  -amax, amax).astype(np.float32)
    return np.ascontiguousarray(
        out.reshape(O, B, OH, OW).transpose(1, 0, 2, 3))


def run_on_hw(x, weight, trace=False):
    from concourse.bass_utils import run_bass_kernel_spmd
    x = np.asarray(x, np.float32)
    weight = np.asarray(weight, np.float32)
    cells = _weight_cells(weight)
    sls = _nz_slices(cells)
    key = tuple(sls)
    if key not in _COMPILED:
        _COMPILED[key] = _build_nc(sls)
    nc = _COMPILED[key]
    in_maps = _make_in_maps(x, weight, sls, cells)
    res = run_bass_kernel_spmd(nc, in_maps, list(range(N_CORES)), trace=trace)
    accs = [np.asarray(res.results[i]["acc_out"], np.float32)
            for i in range(N_CORES)]
    return _postprocess(accs), res


def kernel(x, weight):
    out, _ = run_on_hw(x, weight, trace=False)
    return out
